# revision 67
# baseline (speedup 1.0000x reference)
"""Trainium2 Bass kernel for nn_MessagePassingNN (gnn_message_passing).

B, N, F, H, A, T = 4, 256, 64, 256, 16, 3

Sharding: 8 cores = (batch b, receiver-half). Core c handles batch c//2 and
receiver nodes [128*(c%2), 128*(c%2+1)). All node indexing inside the kernel is
core-RELATIVE ([my 128 | partner 128]) so the SPMD program is identical on all
cores; the host permutes the inputs per core.

Math (per message-passing iteration):
    e[i,j,:] = relu(hi[i,:] + hj[j,:] + b1)           (hi = h@W1_i, hj = h@W1_j)
    agg[i,:] = sum_j adj[i,j] * e[i,j,:] @ W2 + deg[i]*b2     <- linearity trick:
        the W2 matmul is pulled OUT of the j-sum (34 GFLOP -> 134 MFLOP).
    GRU update on agg/h.

Device layout: everything transposed [feature-on-partitions, node-on-free].
Hot loop, [h-partitions, j-free] tiles, bf16 with f32 accumulation:
    1) mask-inject (DVE TT, batched IB receivers via a stride-0 free dim):
       w = hjbT + adj_bc, where adj_bc holds 32*(adj-1) in {-32, 0} so
       masked entries die after the relu (exact for adj=1).
    2) relu+bias+accumulate: one tensor_scalar per (receiver, h-tile):
       (w + hi) max 0.0 with accum_out, split ScalarE(activation)/GpSimd/DVE.
adj_bc is the adjacency row broadcast across 128 partitions (built once by
stride-0 DMAs; iteration-invariant).

Cross-iteration overlap: senders are [my 128 | partner 128]. After GRU(t)
computes hnew, the pair AllReduce of hnew is issued, and while it flies the
kernel computes hjb/hi(t+1) for LOCAL senders plus the local-half injects of
the first K_WIN receiver groups. When the collective lands, the partner half
h is recovered (sum - mine), hjb for remote senders is computed, and the
remaining injects + all accums of iteration t+1 run. The final iteration
exchanges only the [2H,1] readout partial sum instead of the full h.
"""

import sys

sys.path.insert(0, "/opt/trn_rl_repo")

import numpy as np

import concourse.bass as bass
import concourse.bacc as bacc
import concourse.tile as tile
from concourse import mybir
from concourse.bass_utils import run_bass_kernel_spmd

B, N, F, H, A, T = 4, 256, 64, 256, 16, 3
NLOC = 128          # receivers per core
HT = H // 128       # h-dim tiles (2)
f32 = mybir.dt.float32
bf16 = mybir.dt.bfloat16
BF16_NP = mybir.dt.np(bf16)

IB = 8              # receivers per batched mask-inject op
NG = NLOC // IB     # inject groups (16)
# accum engine assignment: within each inject group the IB receivers spread
# across ScalarE (S, activation) / DVE (D, tensor_scalar); S:D = 9:23 per 32.
# (GpSimd has no tensor_scalar opcode; it contributes TT mask-injects.)
# One row per inject group (16). Window groups (0..K_WIN-1) put D at even k so
# their DVE accums can split into local(cc-window)/remote(post-cc) halves.
ENGMAP = [
    "DSDSDSDS",
    "DSDSDSDS",
    "DSDSDSDS",
    "DSDSDSDS",
    "SSDSDSDS",
    "SDSDSDSD",
    "SSDSDSDS",
    "SDSDSDSD",
    "SSDSDSDS",
    "SDSDSDSD",
    "SSDSDSDS",
    "SDSDSDSD",
    "SDSDSDSD",
    "SDSDSDSD",
    "SDSDSDSD",
    "SDSDSDSD",
]
K_WIN = 4           # groups whose local-half inject runs inside the cc window
# GpSimd stays OFF the SBUF-heavy path: its TT adds share the DVE SBUF port
# and halve DVE's 2-port STT throughput (measured 403ns -> 783ns).
GPS_INJ = set()

_CACHE = {}
DEBUG = False       # adds per-iteration dram dumps of aggT / hnew


def _mm_acc(nc, ps, w_sb, m_off, rhs_tiles, kt_count, extra=None):
    """psum ps[:, :] = sum_kt W[kt, m_off:m_off+mw].T @ rhs_tiles[kt]; extra =
    optional (lhsT, rhs) accumulated at the end."""
    n_ins = kt_count + (1 if extra is not None else 0)
    idx = 0
    for kt in range(kt_count):
        lhsT = w_sb[:, kt * w_sb.mcols + m_off: kt * w_sb.mcols + m_off + ps.shape[0]]
        nc.tensor.matmul(ps, lhsT, rhs_tiles[kt], start=(idx == 0), stop=(idx == n_ins - 1))
        idx += 1
    if extra is not None:
        lhsT, rhs = extra
        nc.tensor.matmul(ps, lhsT, rhs, start=False, stop=True)


class _WSb:
    """SBUF weight holder: W [K, M] stored as [128, (K//128)*M]."""

    def __init__(self, nc, pool, dram, K, M, name, dt=f32):
        self.mcols = M
        self.kt = K // 128
        self.sb = pool.tile([128, self.kt * M], dt, name=name, tag=name)
        for kt in range(self.kt):
            nc.sync.dma_start(
                out=self.sb[:, kt * M:(kt + 1) * M],
                in_=dram[kt * 128:(kt + 1) * 128, :],
            )

    def __getitem__(self, sl):
        return self.sb[sl]


def build_program():
    nc = bacc.Bacc("TRN2", target_bir_lowering=False, debug=False, num_devices=8)

    # ---------------- I/O ----------------
    xT_d = nc.dram_tensor("xT", [F, N], f32, kind="ExternalInput")
    adj_d = nc.dram_tensor("adjb", [NLOC, N], bf16, kind="ExternalInput")
    w_pre1 = nc.dram_tensor("pre_W1", [F, H], f32, kind="ExternalInput")
    w_pre2 = nc.dram_tensor("pre_W2", [H, H], f32, kind="ExternalInput")
    w_m1i = nc.dram_tensor("W1i", [H, H], f32, kind="ExternalInput")
    w_m1j = nc.dram_tensor("W1j", [H, H], f32, kind="ExternalInput")
    w_m2 = nc.dram_tensor("W2m", [H, H], f32, kind="ExternalInput")
    w_ih = nc.dram_tensor("Wih", [H, 3 * H], bf16, kind="ExternalInput")
    w_hh = nc.dram_tensor("Whh", [H, 3 * H], bf16, kind="ExternalInput")
    w_ro1 = nc.dram_tensor("roW1", [H, H], f32, kind="ExternalInput")
    w_ro2 = nc.dram_tensor("roW2", [H, A], f32, kind="ExternalInput")
    preb1_d = nc.dram_tensor("preb1c", [128, HT], f32, kind="ExternalInput")
    preb2_d = nc.dram_tensor("preb2c", [128, HT], f32, kind="ExternalInput")
    msgb1_d = nc.dram_tensor("msgb1c", [128, HT], f32, kind="ExternalInput")
    msgb2_d = nc.dram_tensor("msgb2r", [1, H], f32, kind="ExternalInput")
    brz_d = nc.dram_tensor("brzr", [1, 2 * H], bf16, kind="ExternalInput")
    bihn_d = nc.dram_tensor("bihnr", [1, H], bf16, kind="ExternalInput")
    bhhn_d = nc.dram_tensor("bhhnr", [1, H], bf16, kind="ExternalInput")
    rob1_d = nc.dram_tensor("rob1c", [128, HT], f32, kind="ExternalInput")
    rob2_d = nc.dram_tensor("rob2c", [A, 1], f32, kind="ExternalInput")
    ident_d = nc.dram_tensor("ident", [128, 128], f32, kind="ExternalInput")
    q_out = nc.dram_tensor("q_out", [A, 1], f32, kind="ExternalOutput")

    # collective bounce buffers (h exchange per non-final iteration + readout)
    cc_in = [nc.dram_tensor(f"cc_in_{t}", [H, NLOC], bf16) for t in range(T - 1)]
    cc_out = [nc.dram_tensor(f"cc_out_{t}", [H, NLOC], bf16) for t in range(T - 1)]
    ccg_in = nc.dram_tensor("ccg_in", [H, 1], f32)
    ccg_out = nc.dram_tensor("ccg_out", [H, 1], f32)
    dbg = {}
    if DEBUG:
        for t in range(T):
            dbg[f"agg{t}"] = nc.dram_tensor(f"dbg_agg{t}", [H, NLOC], f32, kind="ExternalOutput")
            dbg[f"hnew{t}"] = nc.dram_tensor(f"dbg_hnew{t}", [H, NLOC], f32, kind="ExternalOutput")
            dbg[f"hjb{t}"] = nc.dram_tensor(f"dbg_hjb{t}", [H, N], bf16, kind="ExternalOutput")
    groups = [[0, 1], [2, 3], [4, 5], [6, 7]]

    with tile.TileContext(nc) as tc:
        import contextlib

        with contextlib.ExitStack() as ctx:
            singles = ctx.enter_context(tc.tile_pool(name="singles", bufs=1))
            work = ctx.enter_context(tc.tile_pool(name="work", bufs=3))
            eloop = ctx.enter_context(tc.tile_pool(name="eloop", bufs=4))
            scrp = ctx.enter_context(tc.tile_pool(name="scrp", bufs=10))
            wwin = ctx.enter_context(tc.tile_pool(name="wwin", bufs=2 * K_WIN + 2))
            psp = ctx.enter_context(tc.tile_pool(name="psp", bufs=6, space="PSUM"))

            # ---------------- weights/constants to SBUF ----------------
            # critical path first: preprocess inputs, then the weights, then
            # adjacency broadcast chunks (consumed progressively by iter-0).
            xT_sb = work.tile([F, N], f32, name="xT_sb", tag="xT_sb")
            nc.sync.dma_start(out=xT_sb[:], in_=xT_d[:])
            # pre_W1 has K=F=64 (single sub-128 contraction tile)
            Wpre1_sb = singles.tile([F, H], f32)
            nc.sync.dma_start(out=Wpre1_sb[:], in_=w_pre1[:])
            W_pre2 = _WSb(nc, singles, w_pre2[:], H, H, "Wpre2")
            W_m1i = _WSb(nc, singles, w_m1i[:], H, H, "Wm1i")
            W_m1j = _WSb(nc, singles, w_m1j[:], H, H, "Wm1j")
            adj_sb = singles.tile([NLOC, N], bf16)
            nc.sync.dma_start(out=adj_sb[:], in_=adj_d[:])

            def _load(shape, dram, name, dt=f32):
                t_ = singles.tile(list(shape), dt, name=name, tag=name)
                nc.sync.dma_start(out=t_[:], in_=dram[:])
                return t_

            preb1 = _load([128, HT], preb1_d, "preb1")
            preb2 = _load([128, HT], preb2_d, "preb2")
            msgb1 = _load([128, HT], msgb1_d, "msgb1")
            msgb2 = _load([1, H], msgb2_d, "msgb2")
            ident = _load([128, 128], ident_d, "ident")

            # adjacency rows broadcast across partitions, in iter-0
            # consumption order (chunk c feeds inject groups 2c..2c+1); these
            # precede the GRU/readout weights, which aren't needed until the
            # first e-loop completes.
            CH = 8
            rows_per = NLOC // CH  # 16 receivers per chunk
            gpc = NG // CH         # inject groups per chunk
            adj_bct = [
                singles.tile([128, rows_per * N], bf16, name=f"adjbc{c}", tag=f"adjbc{c}")
                for c in range(CH)
            ]
            for c in range(CH):
                bc_in = bass.AP(
                    tensor=adj_d,
                    offset=c * rows_per * N,
                    ap=[[0, 128], [N, rows_per], [1, N]],
                )
                nc.sync.dma_start(out=adj_bct[c][:], in_=bc_in)

            W_m2 = _WSb(nc, singles, w_m2[:], H, H, "Wm2")
            W_ih = _WSb(nc, singles, w_ih[:], H, 3 * H, "Wih", dt=bf16)
            W_hh = _WSb(nc, singles, w_hh[:], H, 3 * H, "Whh", dt=bf16)
            W_ro1 = _WSb(nc, singles, w_ro1[:], H, H, "Wro1")
            W_ro2 = _WSb(nc, singles, w_ro2[:], H, A, "Wro2")
            brz = _load([1, 2 * H], brz_d, "brz", dt=bf16)
            bihn = _load([1, H], bihn_d, "bihn", dt=bf16)
            bhhn = _load([1, H], bhhn_d, "bhhn", dt=bf16)
            ones_row = singles.tile([1, NLOC], bf16)
            nc.vector.memset(ones_row[:], 1.0)
            rob1 = _load([128, HT], rob1_d, "rob1")
            rob2 = _load([A, 1], rob2_d, "rob2")

            zeros_e = singles.tile([128, N], bf16)
            nc.vector.memset(zeros_e[:], 0.0)

            # degree: adj_sb holds 32*(adj-1) -> deg = reduce/32 + N
            deg_col = singles.tile([NLOC, 1], f32)
            nc.vector.reduce_sum(deg_col[:], adj_sb[:], axis=mybir.AxisListType.X)
            nc.vector.tensor_scalar(
                out=deg_col[:], in0=deg_col[:], scalar1=1.0 / 32.0, scalar2=float(N),
                op0=mybir.AluOpType.mult, op1=mybir.AluOpType.add,
            )
            ps_t = psp.tile([128, 512], f32, name="ps", tag="ps")
            nc.tensor.transpose(ps_t[0:1, 0:NLOC], deg_col[:], ident[:])
            deg_row = singles.tile([1, NLOC], f32)
            nc.vector.tensor_copy(deg_row[:], ps_t[0:1, 0:NLOC])

            # ---------------- preprocess: h0 (all N nodes) ----------------
            hT = [singles.tile([128, N], f32, name=f"hT{ht}", tag=f"hT{ht}") for ht in range(HT)]
            p1 = [work.tile([128, N], f32, name=f"p1_{ht}", tag=f"p1_{ht}") for ht in range(HT)]
            for ht in range(HT):
                ps = psp.tile([128, 512], f32, name="ps", tag="ps")
                nc.tensor.matmul(
                    ps[:, 0:N], Wpre1_sb[:, ht * 128:(ht + 1) * 128], xT_sb[:],
                    start=True, stop=True,
                )
                nc.scalar.activation(
                    p1[ht][:], ps[:, 0:N], mybir.ActivationFunctionType.Relu,
                    bias=preb1[:, ht:ht + 1],
                )
            for ht in range(HT):
                ps = psp.tile([128, 512], f32, name="ps", tag="ps")
                _mm_acc(nc, ps[:, 0:N], W_pre2, ht * 128, p1, HT)
                nc.scalar.activation(
                    hT[ht][:], ps[:, 0:N], mybir.ActivationFunctionType.Identity,
                    bias=preb2[:, ht:ht + 1],
                )

            # ---------------- helpers ----------------
            def emit_accum(i, ht, w_ap, hiTf, out_ap, ncols=N):
                """relu((w) + hi[:, i]) summed over j-cols of w_ap -> out_ap."""
                scr = scrp.tile([128, N], bf16, name="scr", tag="scr")
                e = ENGMAP[(i // IB) % len(ENGMAP)][i % IB]
                if e == "S":
                    nc.scalar.activation(
                        scr[:, 0:ncols], w_ap, mybir.ActivationFunctionType.Relu,
                        bias=hiTf[ht][:, i:i + 1],
                        accum_out=out_ap,
                    )
                else:
                    nc.vector.scalar_tensor_tensor(
                        out=scr[:, 0:ncols], in0=w_ap,
                        scalar=hiTf[ht][:, i:i + 1],
                        in1=zeros_e[:, 0:ncols],
                        op0=mybir.AluOpType.add,
                        op1=mybir.AluOpType.max,
                        accum_out=out_ap,
                    )

            def adj_slice(g, ht_unused, col0, ncols):
                """AP over adj_bct for inject group g, j-cols [col0, col0+ncols)."""
                c, gl = g // gpc, g % gpc
                t_ = adj_bct[c]
                return bass.AP(
                    tensor=t_.tensor, offset=t_.offset + gl * IB * N + col0,
                    ap=[t_.ap[0], [N, IB], [1, ncols]],
                )

            def hjb_rep(tile_, col0, ncols):
                """IB-replicated AP over a [128, >=col0+ncols] hjb tile."""
                return bass.AP(
                    tensor=tile_.tensor, offset=tile_.offset + col0,
                    ap=[tile_.ap[0], [0, IB], [1, ncols]],
                )

            def w_slice(w, col0, ncols):
                return bass.AP(
                    tensor=w.tensor, offset=w.offset + col0,
                    ap=[w.ap[0], [N, IB], [1, ncols]],
                )

            # ---------------- iteration 0 sender prep + e-loop ------------
            # hjbT0 = (h0 @ W1_j + b1).T for all j; hiTf0 = (h0_loc @ W1_i).T
            hTb = [work.tile([128, NLOC], bf16, name=f"hnb{ht}", tag=f"hnb{ht}") for ht in range(HT)]
            for ht in range(HT):
                nc.vector.tensor_copy(hTb[ht][:], hT[ht][:, 0:NLOC])
            hjbT0 = [work.tile([128, N], bf16, name=f"hjbF{ht}", tag=f"hjbF{ht}") for ht in range(HT)]
            for ht in range(HT):
                ps = psp.tile([128, 512], f32, name="ps", tag="ps")
                _mm_acc(nc, ps[:, 0:N], W_m1j, ht * 128, hT, HT)
                nc.scalar.activation(
                    hjbT0[ht][:], ps[:, 0:N],
                    mybir.ActivationFunctionType.Identity,
                    bias=msgb1[:, ht:ht + 1],
                )
            hiTf = [work.tile([128, NLOC], f32, name=f"hiTf{ht}", tag=f"hiTf{ht}") for ht in range(HT)]
            for ht in range(HT):
                ps = psp.tile([128, 512], f32, name="ps", tag="ps")
                _mm_acc(
                    nc, ps[:, 0:NLOC], W_m1i, ht * 128,
                    [h_[:, 0:NLOC] for h_ in hT], HT,
                )
                nc.vector.tensor_copy(hiTf[ht][:], ps[:, 0:NLOC])

            aggT = [work.tile([128, NLOC], f32, name=f"aggT{ht}", tag=f"aggT{ht}") for ht in range(HT)]
            for g in range(NG):
                ieng = nc.gpsimd if g in GPS_INJ else nc.vector
                for ht in range(HT):
                    w = eloop.tile([128, IB * N], bf16, name="w", tag="w")
                    ieng.tensor_tensor(
                        out=w[:], in0=hjb_rep(hjbT0[ht], 0, N),
                        in1=adj_slice(g, ht, 0, N),
                        op=mybir.AluOpType.add,
                    )
                    for k in range(IB):
                        i = g * IB + k
                        emit_accum(i, ht, w[:, k * N:(k + 1) * N], hiTf,
                                   aggT[ht][:, i:i + 1])

            hloc = [h_[:, 0:NLOC] for h_ in hT]
            hlocb = [hTb[ht][:] for ht in range(HT)]
            if DEBUG:
                for ht in range(HT):
                    nc.sync.dma_start(out=dbg["hjb0"][ht * 128:(ht + 1) * 128, :], in_=hjbT0[ht][:])

            # ---------------- iterations ----------------
            for t in range(T):
                if DEBUG:
                    for ht in range(HT):
                        nc.sync.dma_start(out=dbg[f"agg{t}"][ht * 128:(ht + 1) * 128, :], in_=aggT[ht][:])
                # msgT = W2.T @ aggT + outer(b2, deg)
                msgT = [work.tile([128, NLOC], bf16, name=f"msgT{ht}", tag=f"msgT{ht}") for ht in range(HT)]
                for ht in range(HT):
                    ps = psp.tile([128, 512], f32, name="ps", tag="ps")
                    _mm_acc(
                        nc, ps[:, 0:NLOC], W_m2, ht * 128, aggT, HT,
                        extra=(msgb2[0:1, ht * 128:(ht + 1) * 128], deg_row[:]),
                    )
                    nc.vector.tensor_copy(msgT[ht][:], ps[:, 0:NLOC])

                # GRU gates (torch order r, z, n); everything [gate-dim, i].
                # Gate biases ride the matmul chain as rank-1 terms (brz/bihn/
                # bhhn row vectors x ones) so the activations batch bias-free.
                ps_rz = psp.tile([128, 512], f32, name="ps", tag="ps")
                for mt in range(4):  # r0 r1 z0 z1
                    for kt in range(HT):
                        nc.tensor.matmul(
                            ps_rz[:, mt * 128:(mt + 1) * 128],
                            W_ih[:, kt * 768 + mt * 128: kt * 768 + (mt + 1) * 128],
                            msgT[kt][:], start=(kt == 0), stop=False,
                        )
                    for kt in range(HT):
                        nc.tensor.matmul(
                            ps_rz[:, mt * 128:(mt + 1) * 128],
                            W_hh[:, kt * 768 + mt * 128: kt * 768 + (mt + 1) * 128],
                            hlocb[kt], start=False, stop=False,
                        )
                    nc.tensor.matmul(
                        ps_rz[:, mt * 128:(mt + 1) * 128],
                        brz[0:1, mt * 128:(mt + 1) * 128], ones_row[:],
                        start=False, stop=True,
                    )
                rz = work.tile([128, 512], f32, name="rz", tag="rz")
                nc.scalar.activation(
                    rz[:], ps_rz[:], mybir.ActivationFunctionType.Sigmoid,
                )
                # n = tanh(gi_n + bihn + r * (gh_n + bhhn))
                ps_gin = psp.tile([128, 512], f32, name="ps", tag="ps")
                ps_hn = psp.tile([128, 512], f32, name="ps", tag="ps")
                for ht in range(HT):
                    for kt in range(HT):
                        nc.tensor.matmul(
                            ps_gin[:, ht * 128:(ht + 1) * 128],
                            W_ih[:, kt * 768 + (4 + ht) * 128: kt * 768 + (5 + ht) * 128],
                            msgT[kt][:], start=(kt == 0), stop=False,
                        )
                        nc.tensor.matmul(
                            ps_hn[:, ht * 128:(ht + 1) * 128],
                            W_hh[:, kt * 768 + (4 + ht) * 128: kt * 768 + (5 + ht) * 128],
                            hlocb[kt], start=(kt == 0), stop=False,
                        )
                    nc.tensor.matmul(
                        ps_gin[:, ht * 128:(ht + 1) * 128],
                        bihn[0:1, ht * 128:(ht + 1) * 128], ones_row[:],
                        start=False, stop=True,
                    )
                    nc.tensor.matmul(
                        ps_hn[:, ht * 128:(ht + 1) * 128],
                        bhhn[0:1, ht * 128:(ht + 1) * 128], ones_row[:],
                        start=False, stop=True,
                    )
                hnew = [work.tile([128, NLOC], f32, name=f"hnew{ht}", tag=f"hnew{ht}") for ht in range(HT)]
                nsum = work.tile([128, 2 * NLOC], f32, name="nsum", tag="nsum")
                for ht in range(HT):
                    rhn = work.tile([128, NLOC], f32, name="rhn", tag="rhn")
                    nc.vector.tensor_mul(
                        rhn[:], rz[:, ht * 128:(ht + 1) * 128],
                        ps_hn[:, ht * 128:(ht + 1) * 128],
                    )
                    nc.vector.tensor_add(
                        nsum[:, ht * 128:(ht + 1) * 128], rhn[:],
                        ps_gin[:, ht * 128:(ht + 1) * 128],
                    )
                n_t = work.tile([128, 2 * NLOC], f32, name="n_t", tag="n_t")
                nc.scalar.activation(
                    n_t[:], nsum[:], mybir.ActivationFunctionType.Tanh,
                )
                for ht in range(HT):
                    # h' = n + z*(h - n)
                    hmn = work.tile([128, NLOC], f32, name="hmn", tag="hmn")
                    nc.vector.tensor_sub(hmn[:], hloc[ht], n_t[:, ht * 128:(ht + 1) * 128])
                    zh = work.tile([128, NLOC], f32, name="zh", tag="zh")
                    nc.vector.tensor_mul(zh[:], rz[:, 256 + ht * 128: 256 + (ht + 1) * 128], hmn[:])
                    nc.vector.tensor_add(hnew[ht][:], n_t[:, ht * 128:(ht + 1) * 128], zh[:])
                hnewb = [work.tile([128, NLOC], bf16, name=f"hnb{ht}", tag=f"hnb{ht}") for ht in range(HT)]
                for ht in range(HT):
                    nc.vector.tensor_copy(hnewb[ht][:], hnew[ht][:])

                if DEBUG:
                    for ht in range(HT):
                        nc.sync.dma_start(out=dbg[f"hnew{t}"][ht * 128:(ht + 1) * 128, :], in_=hnew[ht][:])

                if t == T - 1:
                    # readout: exchange only the partial node-sum of h_final
                    gpart = [work.tile([128, 1], f32, name=f"gp{ht}", tag=f"gp{ht}") for ht in range(HT)]
                    for ht in range(HT):
                        nc.vector.reduce_sum(gpart[ht][:], hnew[ht][:], axis=mybir.AxisListType.X)
                        nc.sync.dma_start(
                            out=ccg_in[ht * 128:(ht + 1) * 128, :], in_=gpart[ht][:]
                        )
                    nc.gpsimd.collective_compute(
                        "AllReduce", mybir.AluOpType.add, replica_groups=groups,
                        ins=[ccg_in[:]], outs=[ccg_out[:]],
                    )
                    gT = [work.tile([128, 1], f32, name=f"gT{ht}", tag=f"gT{ht}") for ht in range(HT)]
                    for ht in range(HT):
                        nc.sync.dma_start(
                            out=gT[ht][:], in_=ccg_out[ht * 128:(ht + 1) * 128, :]
                        )
                    break

                # start h exchange: AllReduce(pair); partner = sum - mine
                for ht in range(HT):
                    nc.sync.dma_start(
                        out=cc_in[t][ht * 128:(ht + 1) * 128, :], in_=hnewb[ht][:]
                    )
                nc.gpsimd.collective_compute(
                    "AllReduce", mybir.AluOpType.add, replica_groups=groups,
                    ins=[cc_in[t][:]], outs=[cc_out[t][:]],
                )

                # -------- cc window: sender prep (local) + local injects ----
                hjbL = [work.tile([128, NLOC], bf16, name=f"hjbL{ht}", tag=f"hjbL{ht}") for ht in range(HT)]
                hjbF = [work.tile([128, N], bf16, name=f"hjbF{ht}", tag=f"hjbF{ht}") for ht in range(HT)]
                for ht in range(HT):
                    ps = psp.tile([128, 512], f32, name="ps", tag="ps")
                    _mm_acc(nc, ps[:, 0:NLOC], W_m1j, ht * 128, hnew, HT)
                    nc.scalar.activation(
                        hjbL[ht][:], ps[:, 0:NLOC],
                        mybir.ActivationFunctionType.Identity,
                        bias=msgb1[:, ht:ht + 1],
                    )
                    nc.vector.tensor_copy(hjbF[ht][:, 0:NLOC], hjbL[ht][:])
                hiTf = [work.tile([128, NLOC], f32, name=f"hiTf{ht}", tag=f"hiTf{ht}") for ht in range(HT)]
                for ht in range(HT):
                    ps = psp.tile([128, 512], f32, name="ps", tag="ps")
                    _mm_acc(nc, ps[:, 0:NLOC], W_m1i, ht * 128, hnew, HT)
                    nc.vector.tensor_copy(hiTf[ht][:], ps[:, 0:NLOC])

                # window: local-half injects; DVE units also accumulate their
                # local half now (into aggL), finishing post-cc with aggR.
                aggL = [work.tile([128, K_WIN * IB // 2], f32, name=f"aggL{ht}", tag=f"aggL{ht}") for ht in range(HT)]
                aggR = [work.tile([128, K_WIN * IB // 2], f32, name=f"aggR{ht}", tag=f"aggR{ht}") for ht in range(HT)]
                w_win = {}
                for g in range(K_WIN):
                    for ht in range(HT):
                        w = wwin.tile([128, IB * N], bf16, name="ww", tag="ww")
                        nc.vector.tensor_tensor(
                            out=w_slice(w, 0, NLOC), in0=hjb_rep(hjbL[ht], 0, NLOC),
                            in1=adj_slice(g, ht, 0, NLOC),
                            op=mybir.AluOpType.add,
                        )
                        w_win[(g, ht)] = w
                        for k in range(0, IB, 2):  # D units sit at even k
                            i = g * IB + k
                            emit_accum(
                                i, ht, w[:, k * N:k * N + NLOC], hiTf,
                                aggL[ht][:, (g * IB + k) // 2:(g * IB + k) // 2 + 1],
                                ncols=NLOC,
                            )

                # -------- after cc: partner h, remote hjb, finish e-loop ----
                hrem = [work.tile([128, NLOC], f32, name=f"hrem{ht}", tag=f"hrem{ht}") for ht in range(HT)]
                for ht in range(HT):
                    rem = work.tile([128, NLOC], bf16, name="rem", tag="rem")
                    nc.sync.dma_start(
                        out=rem[:], in_=cc_out[t][ht * 128:(ht + 1) * 128, :]
                    )
                    nc.vector.tensor_sub(hrem[ht][:], rem[:], hnewb[ht][:])
                for ht in range(HT):
                    ps = psp.tile([128, 512], f32, name="ps", tag="ps")
                    _mm_acc(nc, ps[:, 0:NLOC], W_m1j, ht * 128, hrem, HT)
                    nc.scalar.activation(
                        hjbF[ht][:, NLOC:N], ps[:, 0:NLOC],
                        mybir.ActivationFunctionType.Identity,
                        bias=msgb1[:, ht:ht + 1],
                    )
                    if DEBUG:
                        nc.sync.dma_start(out=dbg[f"hjb{t + 1}"][ht * 128:(ht + 1) * 128, :], in_=hjbF[ht][:])

                aggT = [work.tile([128, NLOC], f32, name=f"aggT{ht}", tag=f"aggT{ht}") for ht in range(HT)]
                for g in range(NG):
                    for ht in range(HT):
                        if g < K_WIN:
                            w = w_win[(g, ht)]
                            nc.vector.tensor_tensor(
                                out=w_slice(w, NLOC, NLOC),
                                in0=hjb_rep(hjbF[ht], NLOC, NLOC),
                                in1=adj_slice(g, ht, NLOC, NLOC),
                                op=mybir.AluOpType.add,
                            )
                            for k in range(IB):
                                i = g * IB + k
                                if k % 2 == 0:  # D: finish with the remote half
                                    emit_accum(
                                        i, ht, w[:, k * N + NLOC:(k + 1) * N], hiTf,
                                        aggR[ht][:, i // 2:i // 2 + 1], ncols=NLOC,
                                    )
                                else:
                                    emit_accum(i, ht, w[:, k * N:(k + 1) * N], hiTf,
                                               aggT[ht][:, i:i + 1])
                        else:
                            w = eloop.tile([128, IB * N], bf16, name="w", tag="w")
                            ieng = nc.gpsimd if g in GPS_INJ else nc.vector
                            ieng.tensor_tensor(
                                out=w[:], in0=hjb_rep(hjbF[ht], 0, N),
                                in1=adj_slice(g, ht, 0, N),
                                op=mybir.AluOpType.add,
                            )
                            for k in range(IB):
                                i = g * IB + k
                                emit_accum(i, ht, w[:, k * N:(k + 1) * N], hiTf,
                                           aggT[ht][:, i:i + 1])

                # merge the split units: aggT[:, even cols of window range]
                for ht in range(HT):
                    out_ap = bass.AP(
                        tensor=aggT[ht].tensor, offset=aggT[ht].offset,
                        ap=[aggT[ht].ap[0], [2, K_WIN * IB // 2]],
                    )
                    nc.vector.tensor_tensor(
                        out=out_ap, in0=aggL[ht][:], in1=aggR[ht][:],
                        op=mybir.AluOpType.add,
                    )

                hloc = hnew
                hlocb = hnewb

            # ---------------- readout ----------------
            y1 = [work.tile([128, 1], f32, name=f"y1{ht}", tag=f"y1{ht}") for ht in range(HT)]
            for ht in range(HT):
                ps = psp.tile([128, 512], f32, name="ps", tag="ps")
                _mm_acc(nc, ps[:, 0:1], W_ro1, ht * 128, gT, HT)
                nc.scalar.activation(
                    y1[ht][:], ps[:, 0:1], mybir.ActivationFunctionType.Relu,
                    bias=rob1[:, ht:ht + 1],
                )
            ps_q = psp.tile([128, 512], f32, name="ps", tag="ps")
            for kt in range(HT):
                nc.tensor.matmul(
                    ps_q[0:A, 0:1], W_ro2[:, kt * A:(kt + 1) * A], y1[kt][:],
                    start=(kt == 0), stop=(kt == HT - 1),
                )
            q_sb = work.tile([A, 1], f32, name="q_sb", tag="q_sb")
            nc.scalar.activation(
                q_sb[:], ps_q[0:A, 0:1], mybir.ActivationFunctionType.Identity,
                bias=rob2[:],
            )
            nc.sync.dma_start(out=q_out[:], in_=q_sb[:])

    nc.compile()
    return nc


def _in_maps(inputs):
    nf = np.asarray(inputs["node_features"], np.float32)
    adj = np.asarray(inputs["adjacency"])
    msg_W1 = np.asarray(inputs["msg_W1"], np.float32)
    gbih = np.asarray(inputs["gru_bih"], np.float32)
    gbhh = np.asarray(inputs["gru_bhh"], np.float32)

    def cols(v, nt):  # [nt*128] -> [128, nt] partition-major columns
        return np.ascontiguousarray(np.asarray(v, np.float32).reshape(nt, 128).T)

    shared = {
        "pre_W1": np.asarray(inputs["pre_W1"], np.float32),
        "pre_W2": np.asarray(inputs["pre_W2"], np.float32),
        "W1i": np.ascontiguousarray(msg_W1[:H]),
        "W1j": np.ascontiguousarray(msg_W1[H:]),
        "W2m": np.asarray(inputs["msg_W2"], np.float32),
        "Wih": np.asarray(inputs["gru_Wih"], np.float32).astype(BF16_NP),
        "Whh": np.asarray(inputs["gru_Whh"], np.float32).astype(BF16_NP),
        "roW1": np.asarray(inputs["ro_W1"], np.float32),
        "roW2": np.asarray(inputs["ro_W2"], np.float32),
        "preb1c": cols(inputs["pre_b1"], HT),
        "preb2c": cols(inputs["pre_b2"], HT),
        "msgb1c": cols(inputs["msg_b1"], HT),
        "msgb2r": np.asarray(inputs["msg_b2"], np.float32)[None, :],
        "brzr": (gbih + gbhh)[None, : 2 * H].astype(BF16_NP),
        "bihnr": gbih[None, 2 * H:].astype(BF16_NP),
        "bhhnr": gbhh[None, 2 * H:].astype(BF16_NP),
        "rob1c": cols(inputs["ro_b1"], HT),
        "rob2c": np.asarray(inputs["ro_b2"], np.float32)[:, None],
        "ident": np.eye(128, dtype=np.float32),
    }
    maps = []
    for c in range(8):
        b, half = c // 2, c % 2
        lo, hi = half * NLOC, (half + 1) * NLOC
        perm = np.r_[lo:hi, 0:lo, hi:N]
        m = dict(shared)
        m["xT"] = np.ascontiguousarray(nf[b].T[:, perm])
        m["adjb"] = np.ascontiguousarray(
            ((adj[b, lo:hi][:, perm] - 1) * 32).astype(BF16_NP)
        )
        maps.append(m)
    return maps


def kernel(**inputs) -> np.ndarray:
    if "nc" not in _CACHE:
        _CACHE["nc"] = build_program()
    nc = _CACHE["nc"]
    maps = _in_maps(inputs)
    res = run_bass_kernel_spmd(nc, maps, list(range(8))).results
    q = np.stack([res[2 * b]["q_out"][:, 0] for b in range(B)]).astype(np.float32)
    return q


# revision 69
# speedup vs baseline: 1.1709x; 1.1709x over previous
"""Trainium2 Bass kernel for nn_MessagePassingNN (gnn_message_passing).

B, N, F, H, A, T = 4, 256, 64, 256, 16, 3

Sharding: 8 cores = (batch b, receiver-half). Core c handles batch c//2 and
receiver nodes [128*(c%2), 128*(c%2+1)). All node indexing inside the kernel is
core-RELATIVE ([my 128 | partner 128]) so the SPMD program is identical on all
cores; the host permutes the inputs per core.

Math (per message-passing iteration):
    e[i,j,:] = relu(hi[i,:] + hj[j,:] + b1)           (hi = h@W1_i, hj = h@W1_j)
    agg[i,:] = sum_j adj[i,j] * e[i,j,:] @ W2 + deg[i]*b2     <- linearity trick:
        the W2 matmul is pulled OUT of the j-sum (34 GFLOP -> 134 MFLOP).
    GRU update on agg/h.

Device layout: everything transposed [feature-on-partitions, node-on-free].
Hot loop, [h-partitions, j-free] tiles, bf16 with f32 accumulation:
    1) mask-inject (DVE TT, batched IB receivers via a stride-0 free dim):
       w = hjbT + adj_bc, where adj_bc holds 32*(adj-1) in {-32, 0} so
       masked entries die after the relu (exact for adj=1).
    2) relu+bias+accumulate: one tensor_scalar per (receiver, h-tile):
       (w + hi) max 0.0 with accum_out, split ScalarE(activation)/GpSimd/DVE.
adj_bc is the adjacency row broadcast across 128 partitions (built once by
stride-0 DMAs; iteration-invariant).

Cross-iteration overlap: senders are [my 128 | partner 128]. After GRU(t)
computes hnew, the pair AllReduce of hnew is issued, and while it flies the
kernel computes hjb/hi(t+1) for LOCAL senders plus the local-half injects of
the first K_WIN receiver groups. When the collective lands, the partner half
h is recovered (sum - mine), hjb for remote senders is computed, and the
remaining injects + all accums of iteration t+1 run. The final iteration
exchanges only the [2H,1] readout partial sum instead of the full h.
"""

import sys

sys.path.insert(0, "/opt/trn_rl_repo")

import numpy as np

import concourse.bass as bass
import concourse.bacc as bacc
import concourse.tile as tile
from concourse import mybir
from concourse.bass_utils import run_bass_kernel_spmd

B, N, F, H, A, T = 4, 256, 64, 256, 16, 3
NLOC = 128          # receivers per core
HT = H // 128       # h-dim tiles (2)
f32 = mybir.dt.float32
bf16 = mybir.dt.bfloat16
BF16_NP = mybir.dt.np(bf16)

IB = 8              # receivers per batched mask-inject op
NG = NLOC // IB     # inject groups (16)
# accum engine assignment: within each inject group the IB receivers spread
# across ScalarE (S, activation) / DVE (D, tensor_scalar); S:D = 9:23 per 32.
# (GpSimd has no tensor_scalar opcode; it contributes TT mask-injects.)
# One row per inject group (16). Window groups (0..K_WIN-1) put D at even k so
# their DVE accums can split into local(cc-window)/remote(post-cc) halves.
ENGMAP = [
    "DSDSDSDS",
    "DSDSDSDS",
    "DSDSDSDS",
    "DSDSDSDS",
    "SSDSDSDS",
    "SDSDSDSD",
    "SSDSDSDS",
    "SDSDSDSD",
    "SSDSDSDS",
    "SDSDSDSD",
    "SSDSDSDS",
    "SDSDSDSD",
    "SDSDSDSD",
    "SDSDSDSD",
    "SDSDSDSD",
    "SDSDSDSD",
]
K_WIN = 4           # groups whose local-half inject runs inside the cc window
# GpSimd stays OFF the SBUF-heavy path: its TT adds share the DVE SBUF port
# and halve DVE's 2-port STT throughput (measured 403ns -> 783ns).
GPS_INJ = set()

_CACHE = {}
DEBUG = False       # adds per-iteration dram dumps of aggT / hnew


def _mm_acc(nc, ps, w_sb, m_off, rhs_tiles, kt_count, extra=None):
    """psum ps[:, :] = sum_kt W[kt, m_off:m_off+mw].T @ rhs_tiles[kt]; extra =
    optional (lhsT, rhs) accumulated at the end."""
    n_ins = kt_count + (1 if extra is not None else 0)
    idx = 0
    for kt in range(kt_count):
        lhsT = w_sb[:, kt * w_sb.mcols + m_off: kt * w_sb.mcols + m_off + ps.shape[0]]
        nc.tensor.matmul(ps, lhsT, rhs_tiles[kt], start=(idx == 0), stop=(idx == n_ins - 1))
        idx += 1
    if extra is not None:
        lhsT, rhs = extra
        nc.tensor.matmul(ps, lhsT, rhs, start=False, stop=True)


class _WSb:
    """SBUF weight holder: W [K, M] stored as [128, (K//128)*M]."""

    def __init__(self, nc, pool, dram, K, M, name, dt=f32):
        self.mcols = M
        self.kt = K // 128
        self.sb = pool.tile([128, self.kt * M], dt, name=name, tag=name)
        for kt in range(self.kt):
            nc.sync.dma_start(
                out=self.sb[:, kt * M:(kt + 1) * M],
                in_=dram[kt * 128:(kt + 1) * 128, :],
            )

    def __getitem__(self, sl):
        return self.sb[sl]


def build_program():
    nc = bacc.Bacc("TRN2", target_bir_lowering=False, debug=False, num_devices=8)

    # ---------------- I/O ----------------
    xT_d = nc.dram_tensor("xT", [F, N], f32, kind="ExternalInput")
    adj_d = nc.dram_tensor("adjb", [NLOC, N], bf16, kind="ExternalInput")
    w_pre1 = nc.dram_tensor("pre_W1", [F, H], f32, kind="ExternalInput")
    w_pre2 = nc.dram_tensor("pre_W2", [H, H], f32, kind="ExternalInput")
    w_m1i = nc.dram_tensor("W1i", [H, H], f32, kind="ExternalInput")
    w_m1j = nc.dram_tensor("W1j", [H, H], f32, kind="ExternalInput")
    w_m2 = nc.dram_tensor("W2m", [H, H], f32, kind="ExternalInput")
    w_ih = nc.dram_tensor("Wih", [H, 3 * H], bf16, kind="ExternalInput")
    w_hh = nc.dram_tensor("Whh", [H, 3 * H], bf16, kind="ExternalInput")
    w_ro1 = nc.dram_tensor("roW1", [H, H], f32, kind="ExternalInput")
    w_ro2 = nc.dram_tensor("roW2", [H, A], f32, kind="ExternalInput")
    preb1_d = nc.dram_tensor("preb1c", [128, HT], f32, kind="ExternalInput")
    preb2_d = nc.dram_tensor("preb2c", [128, HT], f32, kind="ExternalInput")
    msgb1_d = nc.dram_tensor("msgb1c", [128, HT], f32, kind="ExternalInput")
    msgb2_d = nc.dram_tensor("msgb2r", [1, H], f32, kind="ExternalInput")
    brz_d = nc.dram_tensor("brzr", [1, 2 * H], bf16, kind="ExternalInput")
    bihn_d = nc.dram_tensor("bihnr", [1, H], bf16, kind="ExternalInput")
    bhhn_d = nc.dram_tensor("bhhnr", [1, H], bf16, kind="ExternalInput")
    rob1_d = nc.dram_tensor("rob1c", [128, HT], f32, kind="ExternalInput")
    rob2_d = nc.dram_tensor("rob2c", [A, 1], f32, kind="ExternalInput")
    ident_d = nc.dram_tensor("ident", [128, 128], f32, kind="ExternalInput")
    q_out = nc.dram_tensor("q_out", [A, 1], f32, kind="ExternalOutput")

    # collective bounce buffers (h exchange per non-final iteration + readout)
    cc_in = [nc.dram_tensor(f"cc_in_{t}", [H, NLOC], bf16) for t in range(T - 1)]
    cc_out = [nc.dram_tensor(f"cc_out_{t}", [H, NLOC], bf16) for t in range(T - 1)]
    ccg_in = nc.dram_tensor("ccg_in", [H, 1], f32)
    ccg_out = nc.dram_tensor("ccg_out", [H, 1], f32)
    dbg = {}
    if DEBUG:
        for t in range(T):
            dbg[f"agg{t}"] = nc.dram_tensor(f"dbg_agg{t}", [H, NLOC], f32, kind="ExternalOutput")
            dbg[f"hnew{t}"] = nc.dram_tensor(f"dbg_hnew{t}", [H, NLOC], f32, kind="ExternalOutput")
            dbg[f"hjb{t}"] = nc.dram_tensor(f"dbg_hjb{t}", [H, N], bf16, kind="ExternalOutput")
    groups = [[0, 1], [2, 3], [4, 5], [6, 7]]

    with tile.TileContext(nc) as tc:
        import contextlib

        with contextlib.ExitStack() as ctx:
            singles = ctx.enter_context(tc.tile_pool(name="singles", bufs=1))
            work = ctx.enter_context(tc.tile_pool(name="work", bufs=3))
            eloop = ctx.enter_context(tc.tile_pool(name="eloop", bufs=4))
            scrp = ctx.enter_context(tc.tile_pool(name="scrp", bufs=10))
            wwin = ctx.enter_context(tc.tile_pool(name="wwin", bufs=2 * K_WIN + 2))
            psp = ctx.enter_context(tc.tile_pool(name="psp", bufs=6, space="PSUM"))

            # ---------------- weights/constants to SBUF ----------------
            # critical path first: preprocess inputs, then the weights, then
            # adjacency broadcast chunks (consumed progressively by iter-0).
            xT_sb = work.tile([F, N], f32, name="xT_sb", tag="xT_sb")
            nc.sync.dma_start(out=xT_sb[:], in_=xT_d[:])
            # pre_W1 has K=F=64 (single sub-128 contraction tile)
            Wpre1_sb = singles.tile([F, H], f32)
            nc.sync.dma_start(out=Wpre1_sb[:], in_=w_pre1[:])
            W_pre2 = _WSb(nc, singles, w_pre2[:], H, H, "Wpre2")
            W_m1i = _WSb(nc, singles, w_m1i[:], H, H, "Wm1i")
            W_m1j = _WSb(nc, singles, w_m1j[:], H, H, "Wm1j")
            adj_sb = singles.tile([NLOC, N], bf16)
            nc.sync.dma_start(out=adj_sb[:], in_=adj_d[:])

            def _load(shape, dram, name, dt=f32):
                t_ = singles.tile(list(shape), dt, name=name, tag=name)
                nc.sync.dma_start(out=t_[:], in_=dram[:])
                return t_

            preb1 = _load([128, HT], preb1_d, "preb1")
            preb2 = _load([128, HT], preb2_d, "preb2")
            msgb1 = _load([128, HT], msgb1_d, "msgb1")
            msgb2 = _load([1, H], msgb2_d, "msgb2")
            ident = _load([128, 128], ident_d, "ident")

            # adjacency rows broadcast across partitions, in iter-0
            # consumption order (chunk c feeds inject groups 2c..2c+1); these
            # precede the GRU/readout weights, which aren't needed until the
            # first e-loop completes.
            CH = 8
            rows_per = NLOC // CH  # 16 receivers per chunk
            gpc = NG // CH         # inject groups per chunk
            adj_bct = [
                singles.tile([128, rows_per * N], bf16, name=f"adjbc{c}", tag=f"adjbc{c}")
                for c in range(CH)
            ]
            for c in range(CH):
                bc_in = bass.AP(
                    tensor=adj_d,
                    offset=c * rows_per * N,
                    ap=[[0, 128], [N, rows_per], [1, N]],
                )
                nc.sync.dma_start(out=adj_bct[c][:], in_=bc_in)

            W_m2 = _WSb(nc, singles, w_m2[:], H, H, "Wm2")
            W_ih = _WSb(nc, singles, w_ih[:], H, 3 * H, "Wih", dt=bf16)
            W_hh = _WSb(nc, singles, w_hh[:], H, 3 * H, "Whh", dt=bf16)
            W_ro1 = _WSb(nc, singles, w_ro1[:], H, H, "Wro1")
            W_ro2 = _WSb(nc, singles, w_ro2[:], H, A, "Wro2")
            brz = _load([1, 2 * H], brz_d, "brz", dt=bf16)
            bihn = _load([1, H], bihn_d, "bihn", dt=bf16)
            bhhn = _load([1, H], bhhn_d, "bhhn", dt=bf16)
            ones_row = singles.tile([1, NLOC], bf16)
            nc.vector.memset(ones_row[:], 1.0)
            rob1 = _load([128, HT], rob1_d, "rob1")
            rob2 = _load([A, 1], rob2_d, "rob2")

            zeros_e = singles.tile([128, N], bf16)
            nc.vector.memset(zeros_e[:], 0.0)

            # degree: adj_sb holds 32*(adj-1) -> deg = reduce/32 + N
            deg_col = singles.tile([NLOC, 1], f32)
            nc.vector.reduce_sum(deg_col[:], adj_sb[:], axis=mybir.AxisListType.X)
            nc.vector.tensor_scalar(
                out=deg_col[:], in0=deg_col[:], scalar1=1.0 / 32.0, scalar2=float(N),
                op0=mybir.AluOpType.mult, op1=mybir.AluOpType.add,
            )
            ps_t = psp.tile([128, 512], f32, name="ps", tag="ps")
            nc.tensor.transpose(ps_t[0:1, 0:NLOC], deg_col[:], ident[:])
            deg_row = singles.tile([1, NLOC], f32)
            nc.vector.tensor_copy(deg_row[:], ps_t[0:1, 0:NLOC])

            # ---------------- preprocess: h0 (all N nodes) ----------------
            hT = [singles.tile([128, N], f32, name=f"hT{ht}", tag=f"hT{ht}") for ht in range(HT)]
            p1 = [work.tile([128, N], f32, name=f"p1_{ht}", tag=f"p1_{ht}") for ht in range(HT)]
            for ht in range(HT):
                ps = psp.tile([128, 512], f32, name="ps", tag="ps")
                nc.tensor.matmul(
                    ps[:, 0:N], Wpre1_sb[:, ht * 128:(ht + 1) * 128], xT_sb[:],
                    start=True, stop=True,
                )
                nc.scalar.activation(
                    p1[ht][:], ps[:, 0:N], mybir.ActivationFunctionType.Relu,
                    bias=preb1[:, ht:ht + 1],
                )
            for ht in range(HT):
                ps = psp.tile([128, 512], f32, name="ps", tag="ps")
                _mm_acc(nc, ps[:, 0:N], W_pre2, ht * 128, p1, HT)
                nc.scalar.activation(
                    hT[ht][:], ps[:, 0:N], mybir.ActivationFunctionType.Identity,
                    bias=preb2[:, ht:ht + 1],
                )

            # ---------------- helpers ----------------
            def emit_accum(i, ht, w_ap, hiTf, out_ap, ncols=N):
                """relu((w) + hi[:, i]) summed over j-cols of w_ap -> out_ap."""
                scr = scrp.tile([128, N], bf16, name="scr", tag="scr")
                e = ENGMAP[(i // IB) % len(ENGMAP)][i % IB]
                if e == "S":
                    nc.scalar.activation(
                        scr[:, 0:ncols], w_ap, mybir.ActivationFunctionType.Relu,
                        bias=hiTf[ht][:, i:i + 1],
                        accum_out=out_ap,
                    )
                else:
                    nc.vector.scalar_tensor_tensor(
                        out=scr[:, 0:ncols], in0=w_ap,
                        scalar=hiTf[ht][:, i:i + 1],
                        in1=zeros_e[:, 0:ncols],
                        op0=mybir.AluOpType.add,
                        op1=mybir.AluOpType.max,
                        accum_out=out_ap,
                    )

            def adj_slice(g, ht_unused, col0, ncols):
                """AP over adj_bct for inject group g, j-cols [col0, col0+ncols)."""
                c, gl = g // gpc, g % gpc
                t_ = adj_bct[c]
                return bass.AP(
                    tensor=t_.tensor, offset=t_.offset + gl * IB * N + col0,
                    ap=[t_.ap[0], [N, IB], [1, ncols]],
                )

            def hjb_rep(tile_, col0, ncols):
                """IB-replicated AP over a [128, >=col0+ncols] hjb tile."""
                return bass.AP(
                    tensor=tile_.tensor, offset=tile_.offset + col0,
                    ap=[tile_.ap[0], [0, IB], [1, ncols]],
                )

            def w_slice(w, col0, ncols):
                return bass.AP(
                    tensor=w.tensor, offset=w.offset + col0,
                    ap=[w.ap[0], [N, IB], [1, ncols]],
                )

            # ---------------- iteration 0 sender prep + e-loop ------------
            # hjbT0 = (h0 @ W1_j + b1).T for all j; hiTf0 = (h0_loc @ W1_i).T
            hTb = [work.tile([128, NLOC], bf16, name=f"hnb{ht}", tag=f"hnb{ht}") for ht in range(HT)]
            for ht in range(HT):
                nc.vector.tensor_copy(hTb[ht][:], hT[ht][:, 0:NLOC])
            hjbT0 = [work.tile([128, N], bf16, name=f"hjbF{ht}", tag=f"hjbF{ht}") for ht in range(HT)]
            for ht in range(HT):
                ps = psp.tile([128, 512], f32, name="ps", tag="ps")
                _mm_acc(nc, ps[:, 0:N], W_m1j, ht * 128, hT, HT)
                nc.scalar.activation(
                    hjbT0[ht][:], ps[:, 0:N],
                    mybir.ActivationFunctionType.Identity,
                    bias=msgb1[:, ht:ht + 1],
                )
            hiTf = [work.tile([128, NLOC], f32, name=f"hiTf{ht}", tag=f"hiTf{ht}") for ht in range(HT)]
            for ht in range(HT):
                ps = psp.tile([128, 512], f32, name="ps", tag="ps")
                _mm_acc(
                    nc, ps[:, 0:NLOC], W_m1i, ht * 128,
                    [h_[:, 0:NLOC] for h_ in hT], HT,
                )
                nc.vector.tensor_copy(hiTf[ht][:], ps[:, 0:NLOC])

            aggT = [work.tile([128, NLOC], f32, name=f"aggT{ht}", tag=f"aggT{ht}") for ht in range(HT)]
            for g in range(NG):
                ieng = nc.gpsimd if g in GPS_INJ else nc.vector
                for ht in range(HT):
                    w = eloop.tile([128, IB * N], bf16, name="w", tag="w")
                    ieng.tensor_tensor(
                        out=w[:], in0=hjb_rep(hjbT0[ht], 0, N),
                        in1=adj_slice(g, ht, 0, N),
                        op=mybir.AluOpType.add,
                    )
                    for k in range(IB):
                        i = g * IB + k
                        emit_accum(i, ht, w[:, k * N:(k + 1) * N], hiTf,
                                   aggT[ht][:, i:i + 1])

            hloc = [h_[:, 0:NLOC] for h_ in hT]
            hlocb = [hTb[ht][:] for ht in range(HT)]
            if DEBUG:
                for ht in range(HT):
                    nc.sync.dma_start(out=dbg["hjb0"][ht * 128:(ht + 1) * 128, :], in_=hjbT0[ht][:])

            # ---------------- iterations ----------------
            for t in range(T):
                if DEBUG:
                    for ht in range(HT):
                        nc.sync.dma_start(out=dbg[f"agg{t}"][ht * 128:(ht + 1) * 128, :], in_=aggT[ht][:])
                # msgT = W2.T @ aggT + outer(b2, deg)
                msgT = [work.tile([128, NLOC], bf16, name=f"msgT{ht}", tag=f"msgT{ht}") for ht in range(HT)]
                for ht in range(HT):
                    ps = psp.tile([128, 512], f32, name="ps", tag="ps")
                    _mm_acc(
                        nc, ps[:, 0:NLOC], W_m2, ht * 128, aggT, HT,
                        extra=(msgb2[0:1, ht * 128:(ht + 1) * 128], deg_row[:]),
                    )
                    nc.vector.tensor_copy(msgT[ht][:], ps[:, 0:NLOC])

                # GRU gates (torch order r, z, n); everything [gate-dim, i].
                # Gate biases ride the matmul chain as rank-1 terms (brz/bihn/
                # bhhn row vectors x ones) so the activations batch bias-free.
                ps_rz = psp.tile([128, 512], f32, name="ps", tag="ps")
                for mt in range(4):  # r0 r1 z0 z1
                    for kt in range(HT):
                        nc.tensor.matmul(
                            ps_rz[:, mt * 128:(mt + 1) * 128],
                            W_ih[:, kt * 768 + mt * 128: kt * 768 + (mt + 1) * 128],
                            msgT[kt][:], start=(kt == 0), stop=False,
                        )
                    for kt in range(HT):
                        nc.tensor.matmul(
                            ps_rz[:, mt * 128:(mt + 1) * 128],
                            W_hh[:, kt * 768 + mt * 128: kt * 768 + (mt + 1) * 128],
                            hlocb[kt], start=False, stop=False,
                        )
                    nc.tensor.matmul(
                        ps_rz[:, mt * 128:(mt + 1) * 128],
                        brz[0:1, mt * 128:(mt + 1) * 128], ones_row[:],
                        start=False, stop=True,
                    )
                rz = work.tile([128, 512], f32, name="rz", tag="rz")
                nc.scalar.activation(
                    rz[:], ps_rz[:], mybir.ActivationFunctionType.Sigmoid,
                )
                # n = tanh(gi_n + bihn + r * (gh_n + bhhn))
                ps_gin = psp.tile([128, 512], f32, name="ps", tag="ps")
                ps_hn = psp.tile([128, 512], f32, name="ps", tag="ps")
                for ht in range(HT):
                    for kt in range(HT):
                        nc.tensor.matmul(
                            ps_gin[:, ht * 128:(ht + 1) * 128],
                            W_ih[:, kt * 768 + (4 + ht) * 128: kt * 768 + (5 + ht) * 128],
                            msgT[kt][:], start=(kt == 0), stop=False,
                        )
                        nc.tensor.matmul(
                            ps_hn[:, ht * 128:(ht + 1) * 128],
                            W_hh[:, kt * 768 + (4 + ht) * 128: kt * 768 + (5 + ht) * 128],
                            hlocb[kt], start=(kt == 0), stop=False,
                        )
                    nc.tensor.matmul(
                        ps_gin[:, ht * 128:(ht + 1) * 128],
                        bihn[0:1, ht * 128:(ht + 1) * 128], ones_row[:],
                        start=False, stop=True,
                    )
                    nc.tensor.matmul(
                        ps_hn[:, ht * 128:(ht + 1) * 128],
                        bhhn[0:1, ht * 128:(ht + 1) * 128], ones_row[:],
                        start=False, stop=True,
                    )
                hnew = [work.tile([128, NLOC], f32, name=f"hnew{ht}", tag=f"hnew{ht}") for ht in range(HT)]
                nsum = work.tile([128, 2 * NLOC], f32, name="nsum", tag="nsum")
                for ht in range(HT):
                    rhn = work.tile([128, NLOC], f32, name="rhn", tag="rhn")
                    nc.vector.tensor_mul(
                        rhn[:], rz[:, ht * 128:(ht + 1) * 128],
                        ps_hn[:, ht * 128:(ht + 1) * 128],
                    )
                    nc.vector.tensor_add(
                        nsum[:, ht * 128:(ht + 1) * 128], rhn[:],
                        ps_gin[:, ht * 128:(ht + 1) * 128],
                    )
                n_t = work.tile([128, 2 * NLOC], f32, name="n_t", tag="n_t")
                nc.scalar.activation(
                    n_t[:], nsum[:], mybir.ActivationFunctionType.Tanh,
                )
                for ht in range(HT):
                    # h' = n + z*(h - n)
                    hmn = work.tile([128, NLOC], f32, name="hmn", tag="hmn")
                    nc.vector.tensor_sub(hmn[:], hloc[ht], n_t[:, ht * 128:(ht + 1) * 128])
                    zh = work.tile([128, NLOC], f32, name="zh", tag="zh")
                    nc.vector.tensor_mul(zh[:], rz[:, 256 + ht * 128: 256 + (ht + 1) * 128], hmn[:])
                    nc.vector.tensor_add(hnew[ht][:], n_t[:, ht * 128:(ht + 1) * 128], zh[:])
                hnewb = [work.tile([128, NLOC], bf16, name=f"hnb{ht}", tag=f"hnb{ht}") for ht in range(HT)]
                for ht in range(HT):
                    nc.vector.tensor_copy(hnewb[ht][:], hnew[ht][:])

                if DEBUG:
                    for ht in range(HT):
                        nc.sync.dma_start(out=dbg[f"hnew{t}"][ht * 128:(ht + 1) * 128, :], in_=hnew[ht][:])

                if t == T - 1:
                    # readout: exchange only the partial node-sum of h_final
                    gpart = [work.tile([128, 1], f32, name=f"gp{ht}", tag=f"gp{ht}") for ht in range(HT)]
                    for ht in range(HT):
                        nc.vector.reduce_sum(gpart[ht][:], hnew[ht][:], axis=mybir.AxisListType.X)
                        nc.sync.dma_start(
                            out=ccg_in[ht * 128:(ht + 1) * 128, :], in_=gpart[ht][:]
                        )
                    nc.gpsimd.collective_compute(
                        "AllReduce", mybir.AluOpType.add, replica_groups=groups,
                        ins=[ccg_in[:]], outs=[ccg_out[:]],
                    )
                    gT = [work.tile([128, 1], f32, name=f"gT{ht}", tag=f"gT{ht}") for ht in range(HT)]
                    for ht in range(HT):
                        nc.sync.dma_start(
                            out=gT[ht][:], in_=ccg_out[ht * 128:(ht + 1) * 128, :]
                        )
                    break

                # start h exchange: AllReduce(pair); partner = sum - mine
                for ht in range(HT):
                    nc.sync.dma_start(
                        out=cc_in[t][ht * 128:(ht + 1) * 128, :], in_=hnewb[ht][:]
                    )
                nc.gpsimd.collective_compute(
                    "AllReduce", mybir.AluOpType.add, replica_groups=groups,
                    ins=[cc_in[t][:]], outs=[cc_out[t][:]],
                )

                # -------- cc window: sender prep (local) + local injects ----
                hjbL = [work.tile([128, NLOC], bf16, name=f"hjbL{ht}", tag=f"hjbL{ht}") for ht in range(HT)]
                hjbF = [work.tile([128, N], bf16, name=f"hjbF{ht}", tag=f"hjbF{ht}") for ht in range(HT)]
                for ht in range(HT):
                    ps = psp.tile([128, 512], f32, name="ps", tag="ps")
                    _mm_acc(nc, ps[:, 0:NLOC], W_m1j, ht * 128, hnew, HT)
                    nc.scalar.activation(
                        hjbL[ht][:], ps[:, 0:NLOC],
                        mybir.ActivationFunctionType.Identity,
                        bias=msgb1[:, ht:ht + 1],
                    )
                    nc.vector.tensor_copy(hjbF[ht][:, 0:NLOC], hjbL[ht][:])
                hiTf = [work.tile([128, NLOC], f32, name=f"hiTf{ht}", tag=f"hiTf{ht}") for ht in range(HT)]
                for ht in range(HT):
                    ps = psp.tile([128, 512], f32, name="ps", tag="ps")
                    _mm_acc(nc, ps[:, 0:NLOC], W_m1i, ht * 128, hnew, HT)
                    nc.vector.tensor_copy(hiTf[ht][:], ps[:, 0:NLOC])

                # window: local-half injects; DVE units also accumulate their
                # local half now (into aggL), finishing post-cc with aggR.
                aggL = [work.tile([128, K_WIN * IB // 2], f32, name=f"aggL{ht}", tag=f"aggL{ht}") for ht in range(HT)]
                aggR = [work.tile([128, K_WIN * IB // 2], f32, name=f"aggR{ht}", tag=f"aggR{ht}") for ht in range(HT)]
                w_win = {}
                for g in range(K_WIN):
                    for ht in range(HT):
                        w = wwin.tile([128, IB * N], bf16, name="ww", tag="ww")
                        nc.vector.tensor_tensor(
                            out=w_slice(w, 0, NLOC), in0=hjb_rep(hjbL[ht], 0, NLOC),
                            in1=adj_slice(g, ht, 0, NLOC),
                            op=mybir.AluOpType.add,
                        )
                        w_win[(g, ht)] = w
                        for k in range(0, IB, 2):  # D units sit at even k
                            i = g * IB + k
                            emit_accum(
                                i, ht, w[:, k * N:k * N + NLOC], hiTf,
                                aggL[ht][:, (g * IB + k) // 2:(g * IB + k) // 2 + 1],
                                ncols=NLOC,
                            )

                # -------- after cc: partner h, remote hjb, finish e-loop ----
                hrem = [work.tile([128, NLOC], f32, name=f"hrem{ht}", tag=f"hrem{ht}") for ht in range(HT)]
                for ht in range(HT):
                    rem = work.tile([128, NLOC], bf16, name="rem", tag="rem")
                    nc.sync.dma_start(
                        out=rem[:], in_=cc_out[t][ht * 128:(ht + 1) * 128, :]
                    )
                    nc.vector.tensor_sub(hrem[ht][:], rem[:], hnewb[ht][:])
                for ht in range(HT):
                    ps = psp.tile([128, 512], f32, name="ps", tag="ps")
                    _mm_acc(nc, ps[:, 0:NLOC], W_m1j, ht * 128, hrem, HT)
                    nc.scalar.activation(
                        hjbF[ht][:, NLOC:N], ps[:, 0:NLOC],
                        mybir.ActivationFunctionType.Identity,
                        bias=msgb1[:, ht:ht + 1],
                    )
                    if DEBUG:
                        nc.sync.dma_start(out=dbg[f"hjb{t + 1}"][ht * 128:(ht + 1) * 128, :], in_=hjbF[ht][:])

                aggT = [work.tile([128, NLOC], f32, name=f"aggT{ht}", tag=f"aggT{ht}") for ht in range(HT)]
                for g in range(NG):
                    for ht in range(HT):
                        if g < K_WIN:
                            w = w_win[(g, ht)]
                            nc.vector.tensor_tensor(
                                out=w_slice(w, NLOC, NLOC),
                                in0=hjb_rep(hjbF[ht], NLOC, NLOC),
                                in1=adj_slice(g, ht, NLOC, NLOC),
                                op=mybir.AluOpType.add,
                            )
                            for k in range(IB):
                                i = g * IB + k
                                if k % 2 == 0:  # D: finish with the remote half
                                    emit_accum(
                                        i, ht, w[:, k * N + NLOC:(k + 1) * N], hiTf,
                                        aggR[ht][:, i // 2:i // 2 + 1], ncols=NLOC,
                                    )
                                else:
                                    emit_accum(i, ht, w[:, k * N:(k + 1) * N], hiTf,
                                               aggT[ht][:, i:i + 1])
                        else:
                            w = eloop.tile([128, IB * N], bf16, name="w", tag="w")
                            ieng = nc.gpsimd if g in GPS_INJ else nc.vector
                            ieng.tensor_tensor(
                                out=w[:], in0=hjb_rep(hjbF[ht], 0, N),
                                in1=adj_slice(g, ht, 0, N),
                                op=mybir.AluOpType.add,
                            )
                            for k in range(IB):
                                i = g * IB + k
                                emit_accum(i, ht, w[:, k * N:(k + 1) * N], hiTf,
                                           aggT[ht][:, i:i + 1])

                # merge the split units: aggT[:, even cols of window range]
                for ht in range(HT):
                    out_ap = bass.AP(
                        tensor=aggT[ht].tensor, offset=aggT[ht].offset,
                        ap=[aggT[ht].ap[0], [2, K_WIN * IB // 2]],
                    )
                    nc.vector.tensor_tensor(
                        out=out_ap, in0=aggL[ht][:], in1=aggR[ht][:],
                        op=mybir.AluOpType.add,
                    )

                hloc = hnew
                hlocb = hnewb

            # ---------------- readout ----------------
            y1 = [work.tile([128, 1], f32, name=f"y1{ht}", tag=f"y1{ht}") for ht in range(HT)]
            for ht in range(HT):
                ps = psp.tile([128, 512], f32, name="ps", tag="ps")
                _mm_acc(nc, ps[:, 0:1], W_ro1, ht * 128, gT, HT)
                nc.scalar.activation(
                    y1[ht][:], ps[:, 0:1], mybir.ActivationFunctionType.Relu,
                    bias=rob1[:, ht:ht + 1],
                )
            ps_q = psp.tile([128, 512], f32, name="ps", tag="ps")
            for kt in range(HT):
                nc.tensor.matmul(
                    ps_q[0:A, 0:1], W_ro2[:, kt * A:(kt + 1) * A], y1[kt][:],
                    start=(kt == 0), stop=(kt == HT - 1),
                )
            q_sb = work.tile([A, 1], f32, name="q_sb", tag="q_sb")
            nc.scalar.activation(
                q_sb[:], ps_q[0:A, 0:1], mybir.ActivationFunctionType.Identity,
                bias=rob2[:],
            )
            nc.sync.dma_start(out=q_out[:], in_=q_sb[:])

    nc.compile()
    return nc


def _in_maps(inputs):
    nf = np.asarray(inputs["node_features"], np.float32)
    adj = np.asarray(inputs["adjacency"])
    msg_W1 = np.asarray(inputs["msg_W1"], np.float32)
    gbih = np.asarray(inputs["gru_bih"], np.float32)
    gbhh = np.asarray(inputs["gru_bhh"], np.float32)

    def cols(v, nt):  # [nt*128] -> [128, nt] partition-major columns
        return np.ascontiguousarray(np.asarray(v, np.float32).reshape(nt, 128).T)

    shared = {
        "pre_W1": np.asarray(inputs["pre_W1"], np.float32),
        "pre_W2": np.asarray(inputs["pre_W2"], np.float32),
        "W1i": np.ascontiguousarray(msg_W1[:H]),
        "W1j": np.ascontiguousarray(msg_W1[H:]),
        "W2m": np.asarray(inputs["msg_W2"], np.float32),
        "Wih": np.asarray(inputs["gru_Wih"], np.float32).astype(BF16_NP),
        "Whh": np.asarray(inputs["gru_Whh"], np.float32).astype(BF16_NP),
        "roW1": np.asarray(inputs["ro_W1"], np.float32),
        "roW2": np.asarray(inputs["ro_W2"], np.float32),
        "preb1c": cols(inputs["pre_b1"], HT),
        "preb2c": cols(inputs["pre_b2"], HT),
        "msgb1c": cols(inputs["msg_b1"], HT),
        "msgb2r": np.asarray(inputs["msg_b2"], np.float32)[None, :],
        "brzr": (gbih + gbhh)[None, : 2 * H].astype(BF16_NP),
        "bihnr": gbih[None, 2 * H:].astype(BF16_NP),
        "bhhnr": gbhh[None, 2 * H:].astype(BF16_NP),
        "rob1c": cols(inputs["ro_b1"], HT),
        "rob2c": np.asarray(inputs["ro_b2"], np.float32)[:, None],
        "ident": np.eye(128, dtype=np.float32),
    }
    maps = []
    for c in range(8):
        b, half = c // 2, c % 2
        lo, hi = half * NLOC, (half + 1) * NLOC
        perm = np.r_[lo:hi, 0:lo, hi:N]
        m = dict(shared)
        m["xT"] = np.ascontiguousarray(nf[b].T[:, perm])
        m["adjb"] = np.ascontiguousarray(
            ((adj[b, lo:hi][:, perm] - 1) * 32).astype(BF16_NP)
        )
        maps.append(m)
    return maps


def kernel(**inputs) -> np.ndarray:
    if "nc" not in _CACHE:
        _CACHE["nc"] = build_program()
    nc = _CACHE["nc"]
    maps = _in_maps(inputs)
    res = run_bass_kernel_spmd(nc, maps, list(range(8))).results
    q = np.stack([res[2 * b]["q_out"][:, 0] for b in range(B)]).astype(np.float32)
    return q
